# revision 3
# baseline (speedup 1.0000x reference)
"""VGAE (DGL GraphConv x3 + inner-product decoder) on 8 Trainium2 NeuronCores.

Math (reference): h1 = relu(S @ feat @ W1 + b1); mu = S @ h1 @ W2 + b2;
logvar = S @ h1 @ W3 + b3; adj = mu @ mu.T, where S = D_in^-1/2 A D_out^-1/2
(A from an edge list with multi-edges + self loops).

Strategy (nodes row-sharded over 8 cores, 1536 nodes = 12x128 tiles each):
 - project-first: q = (feat @ W1) * n_src  (math-equivalent, 16x less SpMM data)
 - SpMM: per dst-tile, dma_gather (gpsimd SWDGE) pulls the 256B table rows of
   edge sources into SBUF [128 edges x 64f32]; a 0/1 one-hot [128e x 128slots]
   built on DVE (iota row vs dst-slot compare) maps edges to dst rows via PE
   matmul-accumulate in PSUM (f32, exact segment_sum).
 - halo exchange: AllGather of the per-core q / h1*n_src table shards and of
   muT, via collective_compute over internal DRAM.
 - decode: adj[1536,12288] = muT_shard.T @ mu_fullT per core (K=16 matmuls).

Host does only structure preprocessing: degree norms (bincount+rsqrt) and
edge bucketing/packing by dst tile; all FLOPs on feat/h/mu run on device.
"""
import sys

sys.path.insert(0, "/opt/trn_rl_repo")

import numpy as np

import concourse.bass as bass
import concourse.bacc as bacc
import concourse.mybir as mybir
import concourse.tile as tile
from concourse.bass_utils import run_bass_kernel_spmd
from concourse.library_config import mlp as mlp_lib

N = 12288
F_DIM = 512
H1 = 32
H2 = 16
NCORES = 8
NSHARD = N // NCORES          # 1536
TILES = NSHARD // 128         # 12
NTILES_ALL = N // 128         # 96
F32 = mybir.dt.float32
I16 = mybir.dt.int16

_cache = {}


def _build(CC: int):
    """One SPMD Tile program; CC = padded edge-chunks per dst tile."""
    CAP = CC * 128
    nc = bacc.Bacc("TRN2", target_bir_lowering=False, debug=False,
                   num_devices=NCORES)

    feat_s = nc.dram_tensor("feat_s", [NSHARD, F_DIM], F32, kind="ExternalInput")
    w1 = nc.dram_tensor("w1", [F_DIM, H1], F32, kind="ExternalInput")
    w2 = nc.dram_tensor("w2", [H1, H2], F32, kind="ExternalInput")
    w3 = nc.dram_tensor("w3", [H1, H2], F32, kind="ExternalInput")
    b1r = nc.dram_tensor("b1r", [128, H1], F32, kind="ExternalInput")
    b2c = nc.dram_tensor("b2c", [H2, 1], F32, kind="ExternalInput")
    b3c = nc.dram_tensor("b3c", [H2, 1], F32, kind="ExternalInput")
    ident = nc.dram_tensor("ident", [128, 128], F32, kind="ExternalInput")
    iota = nc.dram_tensor("iota", [128, 128], F32, kind="ExternalInput")
    idxp = nc.dram_tensor("idxp", [128, TILES * CC * 8], I16, kind="ExternalInput")
    dlocp = nc.dram_tensor("dlocp", [128, TILES * CC], F32, kind="ExternalInput")
    nsrcp = nc.dram_tensor("nsrcp", [128, TILES], F32, kind="ExternalInput")
    ndstp = nc.dram_tensor("ndstp", [128, TILES], F32, kind="ExternalInput")

    adj_o = nc.dram_tensor("adj_o", [NSHARD, N], F32, kind="ExternalOutput")
    mu_o = nc.dram_tensor("mu_o", [H2, NSHARD], F32, kind="ExternalOutput")
    lv_o = nc.dram_tensor("lv_o", [H2, NSHARD], F32, kind="ExternalOutput")

    rg = [list(range(NCORES))]

    with tile.TileContext(nc) as tc:
        with tc.tile_pool(name="const", bufs=1) as cst, \
             tc.tile_pool(name="dram", bufs=1, space="DRAM") as dpool, \
             tc.tile_pool(name="feat", bufs=2) as featp, \
             tc.tile_pool(name="ft", bufs=6) as ftp, \
             tc.tile_pool(name="small", bufs=4) as smp, \
             tc.tile_pool(name="msg", bufs=2) as msgp, \
             tc.tile_pool(name="onehot", bufs=4) as vp, \
             tc.tile_pool(name="dec", bufs=4) as decp:

            nc.gpsimd.load_library(mlp_lib)

            # ---- resident constants / graph data ----
            ident_sb = cst.tile([128, 128], F32, tag="ident")
            nc.sync.dma_start(out=ident_sb[:], in_=ident[:])
            iota_sb = cst.tile([128, 128], F32, tag="iota")
            nc.sync.dma_start(out=iota_sb[:], in_=iota[:])
            b1_sb = cst.tile([128, H1], F32, tag="b1")
            nc.sync.dma_start(out=b1_sb[:], in_=b1r[:])
            b2_sb = cst.tile([H2, 1], F32, tag="b2")
            nc.sync.dma_start(out=b2_sb[:], in_=b2c[:])
            b3_sb = cst.tile([H2, 1], F32, tag="b3")
            nc.sync.dma_start(out=b3_sb[:], in_=b3c[:])
            w1_sb = cst.tile([128, 4, H1], F32, tag="w1")
            for k in range(4):
                nc.sync.dma_start(out=w1_sb[:, k, :], in_=w1[k * 128:(k + 1) * 128, :])
            w2_sb = cst.tile([H1, H2], F32, tag="w2")
            nc.sync.dma_start(out=w2_sb[:], in_=w2[:])
            w3_sb = cst.tile([H1, H2], F32, tag="w3")
            nc.sync.dma_start(out=w3_sb[:], in_=w3[:])
            idx_sb = cst.tile([128, TILES * CC * 8], I16, tag="idx")
            nc.sync.dma_start(out=idx_sb[:], in_=idxp[:])
            dloc_sb = cst.tile([128, TILES * CC], F32, tag="dloc")
            nc.sync.dma_start(out=dloc_sb[:], in_=dlocp[:])
            nsrc_sb = cst.tile([128, TILES], F32, tag="nsrc")
            nc.sync.dma_start(out=nsrc_sb[:], in_=nsrcp[:])
            ndst_sb = cst.tile([128, TILES], F32, tag="ndst")
            nc.sync.dma_start(out=ndst_sb[:], in_=ndstp[:])

            muT_sb = cst.tile([H2, NSHARD], F32, tag="muT")
            lvT_sb = cst.tile([H2, NSHARD], F32, tag="lvT")
            mufT_sb = cst.tile([H2, N], F32, tag="mufT")

            # ---- internal DRAM ----
            qb = dpool.tile([NSHARD, 64], F32, tag="qb")
            hb = dpool.tile([NSHARD, 64], F32, tag="hb")
            mb = dpool.tile([H2, NSHARD], F32, tag="mb")
            table1 = dpool.tile([N, 64], F32, tag="t1", addr_space="Shared")
            table2 = dpool.tile([N, 64], F32, tag="t2", addr_space="Shared")
            mufull = dpool.tile([NCORES, H2, NSHARD], F32, tag="muf",
                                addr_space="Shared")

            # ======== stage A: q = (feat @ W1) * n_src, allgather -> table1
            with tc.tile_pool(name="psA", bufs=2, space="PSUM") as psA:
                for t in range(TILES):
                    f_sb = featp.tile([128, F_DIM], F32, tag="f")
                    nc.sync.dma_start(out=f_sb[:],
                                      in_=feat_s[t * 128:(t + 1) * 128, :])
                    fts = []
                    for k in range(4):
                        tr_ps = psA.tile([128, 128], F32, tag="tr")
                        nc.tensor.transpose(tr_ps[:], f_sb[:, k * 128:(k + 1) * 128],
                                            ident_sb[:])
                        ft_sb = ftp.tile([128, 128], F32, tag="ft")
                        nc.vector.tensor_copy(ft_sb[:], tr_ps[:])
                        fts.append(ft_sb)
                    q_ps = psA.tile([128, H1], F32, tag="q")
                    for k in range(4):
                        nc.tensor.matmul(q_ps[:], lhsT=fts[k][:], rhs=w1_sb[:, k, :],
                                         start=(k == 0), stop=(k == 3))
                    q_sb = smp.tile([128, H1], F32, tag="q_sb")
                    nc.vector.tensor_scalar(q_sb[:], q_ps[:], nsrc_sb[:, t:t + 1],
                                            None, op0=mybir.AluOpType.mult)
                    nc.sync.dma_start(out=qb[t * 128:(t + 1) * 128, 0:H1],
                                      in_=q_sb[:])
            nc.gpsimd.collective_compute(
                "AllGather", mybir.AluOpType.bypass, ins=[qb.opt()],
                outs=[table1.opt()], replica_groups=rg)

            # ======== stages B/C: two SpMM passes ========
            with tc.tile_pool(name="psC", bufs=2, space="PSUM") as psC:
                # pass 1: h1 = relu(S@q + b1); table2 rows = h1 * n_src
                for t in range(TILES):
                    msg = msgp.tile([128, CC, 64], F32, tag="msg")
                    nc.gpsimd.dma_gather(
                        msg[:], table1[:],
                        idx_sb[:, t * CC * 8:(t + 1) * CC * 8],
                        CAP, CAP, 64, single_packet=False)
                    agg_ps = psC.tile([128, H1], F32, tag="agg")
                    for cc in range(CC):
                        v_sb = vp.tile([128, 128], F32, tag="v")
                        nc.vector.tensor_scalar(
                            v_sb[:], iota_sb[:],
                            dloc_sb[:, t * CC + cc:t * CC + cc + 1],
                            None, op0=mybir.AluOpType.is_equal)
                        nc.tensor.matmul(agg_ps[:], lhsT=v_sb[:],
                                         rhs=msg[:, cc, 0:H1],
                                         start=(cc == 0), stop=(cc == CC - 1))
                    h_sb = smp.tile([128, H1], F32, tag="h")
                    nc.vector.tensor_scalar(h_sb[:], agg_ps[:],
                                            ndst_sb[:, t:t + 1], None,
                                            op0=mybir.AluOpType.mult)
                    nc.vector.tensor_tensor(h_sb[:], h_sb[:], b1_sb[:],
                                            op=mybir.AluOpType.add)
                    h2_sb = smp.tile([128, H1], F32, tag="h2")
                    nc.scalar.activation(h2_sb[:], h_sb[:],
                                         mybir.ActivationFunctionType.Relu)
                    h3_sb = smp.tile([128, H1], F32, tag="h3")
                    nc.vector.tensor_scalar(h3_sb[:], h2_sb[:],
                                            nsrc_sb[:, t:t + 1], None,
                                            op0=mybir.AluOpType.mult)
                    nc.sync.dma_start(out=hb[t * 128:(t + 1) * 128, 0:H1],
                                      in_=h3_sb[:])
                nc.gpsimd.collective_compute(
                    "AllGather", mybir.AluOpType.bypass, ins=[hb.opt()],
                    outs=[table2.opt()], replica_groups=rg)

                # pass 2: g = (S@h1s)*n_dst; muT/lvT = W.T @ g.T + b
                for t in range(TILES):
                    msg = msgp.tile([128, CC, 64], F32, tag="msg")
                    nc.gpsimd.dma_gather(
                        msg[:], table2[:],
                        idx_sb[:, t * CC * 8:(t + 1) * CC * 8],
                        CAP, CAP, 64, single_packet=False)
                    agg_ps = psC.tile([128, H1], F32, tag="agg")
                    for cc in range(CC):
                        v_sb = vp.tile([128, 128], F32, tag="v")
                        nc.vector.tensor_scalar(
                            v_sb[:], iota_sb[:],
                            dloc_sb[:, t * CC + cc:t * CC + cc + 1],
                            None, op0=mybir.AluOpType.is_equal)
                        nc.tensor.matmul(agg_ps[:], lhsT=v_sb[:],
                                         rhs=msg[:, cc, 0:H1],
                                         start=(cc == 0), stop=(cc == CC - 1))
                    g_sb = smp.tile([128, H1], F32, tag="g")
                    nc.vector.tensor_scalar(g_sb[:], agg_ps[:],
                                            ndst_sb[:, t:t + 1], None,
                                            op0=mybir.AluOpType.mult)
                    gt_ps = psC.tile([H1, 128], F32, tag="gt")
                    nc.tensor.transpose(gt_ps[:], g_sb[:], ident_sb[:])
                    gt_sb = smp.tile([H1, 128], F32, tag="gts")
                    nc.vector.tensor_copy(gt_sb[:], gt_ps[:])
                    mu_ps = psC.tile([H2, 128], F32, tag="mu")
                    nc.tensor.matmul(mu_ps[:], lhsT=w2_sb[:], rhs=gt_sb[:],
                                     start=True, stop=True)
                    lv_ps = psC.tile([H2, 128], F32, tag="lv")
                    nc.tensor.matmul(lv_ps[:], lhsT=w3_sb[:], rhs=gt_sb[:],
                                     start=True, stop=True)
                    nc.vector.tensor_scalar(muT_sb[:, t * 128:(t + 1) * 128],
                                            mu_ps[:], b2_sb[:], None,
                                            op0=mybir.AluOpType.add)
                    nc.vector.tensor_scalar(lvT_sb[:, t * 128:(t + 1) * 128],
                                            lv_ps[:], b3_sb[:], None,
                                            op0=mybir.AluOpType.add)

            nc.sync.dma_start(out=mu_o[:], in_=muT_sb[:])
            nc.sync.dma_start(out=lv_o[:], in_=lvT_sb[:])
            nc.sync.dma_start(out=mb[:], in_=muT_sb[:])
            nc.gpsimd.collective_compute(
                "AllGather", mybir.AluOpType.bypass, ins=[mb.opt()],
                outs=[mufull.opt()], replica_groups=rg)
            for r in range(NCORES):
                nc.sync.dma_start(out=mufT_sb[:, r * NSHARD:(r + 1) * NSHARD],
                                  in_=mufull[r, :, :])

            # ======== stage E: adj block = muT_shard.T @ mufT ========
            NBLK = N // 512  # 24
            with tc.tile_pool(name="psE", bufs=4, space="PSUM") as psE:
                for t in range(TILES):
                    for nb in range(NBLK):
                        d_ps = psE.tile([128, 512], F32, tag="d")
                        nc.tensor.matmul(
                            d_ps[:], lhsT=muT_sb[:, t * 128:(t + 1) * 128],
                            rhs=mufT_sb[:, nb * 512:(nb + 1) * 512],
                            start=True, stop=True)
                        d_sb = decp.tile([128, 512], F32, tag="ds")
                        nc.vector.tensor_copy(d_sb[:], d_ps[:])
                        nc.sync.dma_start(
                            out=adj_o[t * 128:(t + 1) * 128,
                                      nb * 512:(nb + 1) * 512],
                            in_=d_sb[:])
    nc.compile()
    return nc


def _preprocess(src, dst):
    deg_out = np.bincount(src, minlength=N).astype(np.float32)
    deg_in = np.bincount(dst, minlength=N).astype(np.float32)
    n_src = np.where(deg_out > 0,
                     1.0 / np.sqrt(np.maximum(deg_out, 1.0)), 0.0).astype(np.float32)
    n_dst = np.where(deg_in > 0,
                     1.0 / np.sqrt(np.maximum(deg_in, 1.0)), 0.0).astype(np.float32)

    tile_id = dst // 128
    order = np.argsort(tile_id, kind="stable")
    s_sorted = src[order].astype(np.int16)
    dl_sorted = (dst[order] % 128).astype(np.float32)
    counts = np.bincount(tile_id, minlength=NTILES_ALL)
    CC = int(np.ceil(counts.max() / 128))
    CAP = CC * 128
    idx_pad = np.zeros((NTILES_ALL, CAP), np.int16)
    dloc_pad = np.full((NTILES_ALL, CAP), -1.0, np.float32)
    offs = np.zeros(NTILES_ALL + 1, np.int64)
    offs[1:] = np.cumsum(counts)
    grp = np.repeat(np.arange(NTILES_ALL), counts)
    pos = np.arange(len(order)) - offs[grp]
    idx_pad[grp, pos] = s_sorted
    dloc_pad[grp, pos] = dl_sorted
    return n_src, n_dst, idx_pad, dloc_pad, CC, CAP


def kernel(feat, src, dst, W1, b1, W2, b2, W3, b3):
    feat = np.ascontiguousarray(np.asarray(feat, np.float32))
    src = np.asarray(src, np.int64)
    dst = np.asarray(dst, np.int64)
    W1 = np.asarray(W1, np.float32)
    W2 = np.asarray(W2, np.float32)
    W3 = np.asarray(W3, np.float32)
    b1 = np.asarray(b1, np.float32)
    b2 = np.asarray(b2, np.float32)
    b3 = np.asarray(b3, np.float32)

    n_src, n_dst, idx_pad, dloc_pad, CC, CAP = _preprocess(src, dst)

    if CC not in _cache:
        _cache[CC] = _build(CC)
    nc = _cache[CC]

    ident = np.eye(128, dtype=np.float32)
    iota = np.broadcast_to(np.arange(128, dtype=np.float32),
                           (128, 128)).copy()
    b1r = np.broadcast_to(b1, (128, H1)).copy()
    b2c = np.ascontiguousarray(b2.reshape(H2, 1))
    b3c = np.ascontiguousarray(b3.reshape(H2, 1))

    in_maps = []
    for c in range(NCORES):
        tsel = slice(c * TILES, (c + 1) * TILES)
        # gather indices: per tile [16, CAP//16] (idx i -> [i%16, i//16]),
        # tiles concatenated along cols, replicated over the 8 Q7 groups
        w = idx_pad[tsel].reshape(TILES, CAP // 16, 16).transpose(2, 0, 1) \
            .reshape(16, TILES * (CAP // 16))
        idxp = np.ascontiguousarray(np.tile(w, (8, 1)))
        # dst slots: per tile [128, CC] (edge i -> [i%128, i//128])
        dlocp = np.ascontiguousarray(
            dloc_pad[tsel].reshape(TILES, CC, 128).transpose(2, 0, 1)
            .reshape(128, TILES * CC))
        nsrcp = np.ascontiguousarray(
            n_src[c * NSHARD:(c + 1) * NSHARD].reshape(TILES, 128).T)
        ndstp = np.ascontiguousarray(
            n_dst[c * NSHARD:(c + 1) * NSHARD].reshape(TILES, 128).T)
        in_maps.append({
            "feat_s": feat[c * NSHARD:(c + 1) * NSHARD],
            "w1": W1, "w2": W2, "w3": W3,
            "b1r": b1r, "b2c": b2c, "b3c": b3c,
            "ident": ident, "iota": iota,
            "idxp": idxp, "dlocp": dlocp,
            "nsrcp": nsrcp, "ndstp": ndstp,
        })

    global _last_in_maps
    _last_in_maps = in_maps
    res = run_bass_kernel_spmd(nc, in_maps, list(range(NCORES)))

    adj = np.empty((N, N), np.float32)
    mu = np.empty((N, H2), np.float32)
    logvar = np.empty((N, H2), np.float32)
    for c in range(NCORES):
        r = res.results[c]
        adj[c * NSHARD:(c + 1) * NSHARD] = r["adj_o"]
        mu[c * NSHARD:(c + 1) * NSHARD] = r["mu_o"].T
        logvar[c * NSHARD:(c + 1) * NSHARD] = r["lv_o"].T
    return adj, mu, logvar
